# revision 6
# baseline (speedup 1.0000x reference)
"""Causal attention (single head, d=1024) on 8 trn2 NeuronCores.

Problem: x[4,2048,1024], Wq/Wk/Wv[1024,1024] fp32;
out = softmax(mask(QK^T)/sqrt(1024)) @ V with mask j <= i+1.

Sharding: 2 cores per batch. Causal row work grows ~linearly with row
index, so the two cores split the 16 row-blocks of 128 as
{g : g%4 in {0,3}} vs {g : g%4 in {1,2}} (balanced). Each core receives
x[b] with its own rows permuted to the front so that every core runs the
same SPMD program; causality is enforced by a per-core additive mask
tensor (data, not code). K/V are computed redundantly per core (v1, no
collectives).

Matmul dtypes: fp32r (TF32-like, full rate at N>=512) for the
precision-critical Q/K/S chain; bf16 for V and P (attention weights).
"""

import numpy as np
import ml_dtypes

import concourse.bass as bass
import concourse.mybir as mybir
import concourse.tile as tile
from concourse import bacc, masks
from concourse.bass_utils import run_bass_kernel_spmd

B, S, D, DA = 4, 2048, 1024, 1024
NCORES = 8
NBLK = S // 128  # 16 row blocks per batch
F32 = mybir.dt.float32
F32R = mybir.dt.float32r
BF16 = mybir.dt.bfloat16

ABLK = [g for g in range(NBLK) if g % 4 in (0, 3)]
BBLK = [g for g in range(NBLK) if g % 4 in (1, 2)]

NEG = -1e30


def _perm_rows(my):
    oth = [g for g in range(NBLK) if g not in my]
    idx = []
    for g in my + oth:
        idx.extend(range(g * 128, (g + 1) * 128))
    return np.array(idx, dtype=np.int64)


def _chunk_schedule():
    """Per local row-block l: which 512-col chunks of the permuted S row
    must be computed (union over the two roles, so the program is SPMD)."""
    sched = []
    for l in range(8):
        need = [False] * 4
        for my in (ABLK, BBLK):
            perm = _perm_rows(my)  # permuted col -> global row
            jmax = my[l] * 128 + 127 + 1  # max attended global col
            attended = perm <= jmax
            for ch in range(4):
                if attended[ch * 512 : (ch + 1) * 512].any():
                    need[ch] = True
        sched.append([ch for ch in range(4) if need[ch]])
    return sched


CHUNKS = _chunk_schedule()

_CACHE = {}


def _build():
    if "nc" in _CACHE:
        return _CACHE["nc"]

    nc = bacc.Bacc()
    x_d = nc.dram_tensor("x_perm", [S, D], F32, kind="ExternalInput")
    wq_d = nc.dram_tensor("wq", [D, DA], F32, kind="ExternalInput")
    wk_d = nc.dram_tensor("wk", [D, DA], F32, kind="ExternalInput")
    wv_d = nc.dram_tensor("wv", [D, DA], F32, kind="ExternalInput")
    mask_d = nc.dram_tensor("maskb", [1024, S], BF16, kind="ExternalInput")
    out_d = nc.dram_tensor("out", [1024, DA], F32, kind="ExternalOutput")
    xt_d = nc.dram_tensor("xt", [D, S], F32)  # internal scratch: x_perm^T

    from contextlib import ExitStack

    with tile.TileContext(nc) as tc, ExitStack() as stack:
        cpool = stack.enter_context(tc.tile_pool(name="const", bufs=1))
        ident = cpool.tile([128, 128], F32, tag="ident")
        masks.make_identity(nc, ident[:])

        # resident accumulators
        respool = stack.enter_context(tc.tile_pool(name="resident", bufs=1))
        QT = [respool.tile([128, 1024], F32R, name=f"qt{a}", tag=f"qt{a}") for a in range(8)]
        KT = [respool.tile([128, S], F32R, name=f"kt{a}", tag=f"kt{a}") for a in range(8)]
        V = [respool.tile([128, DA], BF16, name=f"v{j}", tag=f"v{j}") for j in range(16)]

        # ---- Phase 0: x_perm -> xt (transpose via PE) --------------------
        with (
            tc.tile_pool(name="ph0", bufs=3) as p0,
            tc.tile_pool(name="ph0ps", bufs=2, space="PSUM") as p0ps,
        ):
            for rbg in range(4):  # groups of 4 row-blocks (512 rows)
                xn = [p0.tile([128, D], F32, name=f"xn{i}", tag=f"xn{i}") for i in range(4)]
                for i in range(4):
                    r0 = (rbg * 4 + i) * 128
                    nc.sync.dma_start(xn[i][:], x_d[r0 : r0 + 128, :])
                for dc in range(8):
                    pst = p0ps.tile([128, 512], F32, tag="pst")
                    for i in range(4):
                        nc.tensor.transpose(
                            pst[:, i * 128 : (i + 1) * 128],
                            xn[i][:, dc * 128 : (dc + 1) * 128],
                            ident[:],
                        )
                    xt_sb = p0.tile([128, 512], F32, tag="xt_sb")
                    nc.vector.tensor_copy(xt_sb[:], pst[:])
                    nc.sync.dma_start(
                        xt_d[dc * 128 : (dc + 1) * 128, rbg * 512 : (rbg + 1) * 512],
                        xt_sb[:],
                    )

        # ---- Phase 1: QT, KT, V ------------------------------------------
        def load_w(pool, w_d):
            w = [pool.tile([128, DA], F32R, name=f"w{d}", tag=f"w{d}") for d in range(8)]
            for d in range(8):
                nc.gpsimd.dma_start(w[d][:], w_d[d * 128 : (d + 1) * 128, :])
            return w

        def load_xt(pool, jc):
            xt = [pool.tile([128, 512], F32R, name=f"xtr{d}", tag=f"xtr{d}") for d in range(8)]
            for d in range(8):
                nc.gpsimd.dma_start(
                    xt[d][:], xt_d[d * 128 : (d + 1) * 128, jc * 512 : (jc + 1) * 512]
                )
            return xt

        # Q pass (own cols 0:1024)
        with (
            tc.tile_pool(name="phqw", bufs=1) as pqw,
            tc.tile_pool(name="phq", bufs=2) as pq,
            tc.tile_pool(name="phqps", bufs=4, space="PSUM") as pqps,
        ):
            wq = load_w(pqw, wq_d)
            for jc in range(2):
                xt = load_xt(pq, jc)
                for ac in range(8):
                    ps = pqps.tile([128, 512], F32, tag="ps")
                    for d in range(8):
                        nc.tensor.matmul(
                            ps[:],
                            wq[d][:, ac * 128 : (ac + 1) * 128],
                            xt[d][:],
                            start=(d == 0),
                            stop=(d == 7),
                        )
                    nc.vector.tensor_copy(QT[ac][:, jc * 512 : (jc + 1) * 512], ps[:])

        # K pass (all cols)
        with (
            tc.tile_pool(name="phkw", bufs=1) as pkw,
            tc.tile_pool(name="phk", bufs=2) as pk,
            tc.tile_pool(name="phkps", bufs=4, space="PSUM") as pkps,
        ):
            wk = load_w(pkw, wk_d)
            for jc in range(4):
                xt = load_xt(pk, jc)
                for ac in range(8):
                    ps = pkps.tile([128, 512], F32, tag="ps")
                    for d in range(8):
                        nc.tensor.matmul(
                            ps[:],
                            wk[d][:, ac * 128 : (ac + 1) * 128],
                            xt[d][:],
                            start=(d == 0),
                            stop=(d == 7),
                        )
                    nc.vector.tensor_copy(KT[ac][:, jc * 512 : (jc + 1) * 512], ps[:])

        # V pass (all rows; V[j,da] = sum_d xT[d,j] * Wv[d,da])
        with (
            tc.tile_pool(name="phvw", bufs=1) as pvw,
            tc.tile_pool(name="phv", bufs=2) as pv,
            tc.tile_pool(name="phvps", bufs=4, space="PSUM") as pvps,
        ):
            wv = load_w(pvw, wv_d)
            for jc in range(4):
                xt = load_xt(pv, jc)
                for q in range(4):
                    vj = jc * 4 + q
                    for half in range(2):
                        ps = pvps.tile([128, 512], F32, tag="ps")
                        for d in range(8):
                            nc.tensor.matmul(
                                ps[:],
                                xt[d][:, q * 128 : (q + 1) * 128],
                                wv[d][:, half * 512 : (half + 1) * 512],
                                start=(d == 0),
                                stop=(d == 7),
                            )
                        nc.vector.tensor_copy(
                            V[vj][:, half * 512 : (half + 1) * 512], ps[:]
                        )

        # ---- Phase 2: attention per local row-block ----------------------
        with (
            tc.tile_pool(name="attn", bufs=2) as pa,
            tc.tile_pool(name="attn1", bufs=2) as pa1,
            tc.tile_pool(name="psS", bufs=2, space="PSUM") as psS,
            tc.tile_pool(name="psT", bufs=2, space="PSUM") as psT,
            tc.tile_pool(name="psO", bufs=1, space="PSUM") as psO,
        ):
            for l in range(8):
                chunks = CHUNKS[l]
                nch = len(chunks)
                W = nch * 512
                S_sb = pa.tile([128, 2048], F32, tag="S")
                for k, ch in enumerate(chunks):
                    ps = psS.tile([128, 512], F32, tag="ps")
                    for ac in range(8):
                        nc.tensor.matmul(
                            ps[:],
                            QT[ac][:, l * 128 : (l + 1) * 128],
                            KT[ac][:, ch * 512 : (ch + 1) * 512],
                            start=(ac == 0),
                            stop=(ac == 7),
                        )
                    mk = pa1.tile([128, 512], BF16, tag="mk")
                    nc.sync.dma_start(
                        mk[:], mask_d[l * 128 : (l + 1) * 128, ch * 512 : (ch + 1) * 512]
                    )
                    nc.vector.tensor_add(S_sb[:, k * 512 : (k + 1) * 512], ps[:], mk[:])

                mx = pa1.tile([128, 1], F32, tag="mx")
                nc.vector.reduce_max(mx[:], S_sb[:, 0:W], axis=mybir.AxisListType.X)
                negb = pa1.tile([128, 1], F32, tag="negb")
                nc.vector.tensor_scalar_mul(negb[:], mx[:], -1.0 / 32.0)
                P_sb = pa.tile([128, 2048], F32, tag="P")
                rs = pa1.tile([128, 1], F32, tag="rs")
                nc.scalar.activation(
                    P_sb[:, 0:W],
                    S_sb[:, 0:W],
                    mybir.ActivationFunctionType.Exp,
                    bias=negb[:],
                    scale=1.0 / 32.0,
                    accum_out=rs[:],
                )

                oacc = [psO.tile([128, 512], F32, name=f"oacc{h}", tag=f"oacc{h}") for h in range(2)]
                nq = nch * 4
                for q in range(nq):
                    vj = chunks[q // 4] * 4 + (q % 4)
                    pst = psT.tile([128, 128], F32, tag="pst")
                    nc.tensor.transpose(
                        pst[:], P_sb[:, q * 128 : (q + 1) * 128], ident[:]
                    )
                    pt = pa1.tile([128, 128], BF16, tag="pt")
                    nc.vector.tensor_copy(pt[:], pst[:])
                    for half in range(2):
                        nc.tensor.matmul(
                            oacc[half][:],
                            pt[:],
                            V[vj][:, half * 512 : (half + 1) * 512],
                            start=(q == 0),
                            stop=(q == nq - 1),
                        )

                rec = pa1.tile([128, 1], F32, tag="rec")
                nc.vector.reciprocal(rec[:], rs[:])
                for half in range(2):
                    o_sb = pa1.tile([128, 512], F32, tag="o")
                    nc.scalar.activation(
                        o_sb[:],
                        oacc[half][:],
                        mybir.ActivationFunctionType.Copy,
                        bias=0.0,
                        scale=rec[:],
                    )
                    nc.sync.dma_start(
                        out_d[l * 128 : (l + 1) * 128, half * 512 : (half + 1) * 512],
                        o_sb[:],
                    )

    nc.compile()
    _CACHE["nc"] = nc
    return nc


def kernel(x, Wq, Wk, Wv):
    x = np.ascontiguousarray(np.asarray(x, dtype=np.float32))
    Wq = np.ascontiguousarray(np.asarray(Wq, dtype=np.float32))
    Wk = np.ascontiguousarray(np.asarray(Wk, dtype=np.float32))
    Wv = np.ascontiguousarray(np.asarray(Wv, dtype=np.float32))

    nc = _build()

    in_maps = []
    metas = []
    for c in range(NCORES):
        b = c // 2
        my = ABLK if c % 2 == 0 else BBLK
        perm = _perm_rows(my)
        x_perm = x[b][perm]
        gi = np.concatenate(
            [np.arange(g * 128, (g + 1) * 128) for g in my]
        )  # [1024] global row of each local row
        gj = perm  # [2048] global row of each permuted col
        mask = np.where(gj[None, :] <= gi[:, None] + 1, 0.0, NEG).astype(
            ml_dtypes.bfloat16
        )
        in_maps.append(
            {
                "x_perm": x_perm,
                "wq": Wq,
                "wk": Wk,
                "wv": Wv,
                "maskb": mask,
            }
        )
        metas.append((b, my))

    res = run_bass_kernel_spmd(nc, in_maps, list(range(NCORES)))

    out = np.empty((B, S, DA), dtype=np.float32)
    for c in range(NCORES):
        b, my = metas[c]
        o = res.results[c]["out"]
        for l, g in enumerate(my):
            out[b, g * 128 : (g + 1) * 128] = o[l * 128 : (l + 1) * 128]
    return out
